# revision 1
# baseline (speedup 1.0000x reference)
"""Causal multi-head attention with RoPE (faithful to reference's cos<-sin
overwrite bug) on 8 TRN2 NeuronCores.

Sharding: data parallel on batch (2) x tensor parallel on heads (4 groups of
4 heads) = 8 cores. Each core computes, for its (batch, 4 heads):
  flash-style causal attention in transposed layout, then its partial
  out-projection z_partial = concat_h(O_h) @ Wo[head rows].
Host sums the 4 partials per batch and adds the bias.

Device kernel math notes:
- The rotate-half of RoPE is a linear map R on the head dim, folded into the
  Q/K projection weights on the host (Wq' = Wq @ R.T per head block), so the
  on-device rope is a single elementwise multiply by a sin table.
- Q^T/K^T are produced directly in [head_dim, ctx] layout (feature-major)
  from x^T, which the host passes pre-transposed in bf16.
- Scores are computed transposed (S^T[k, q]), exp'd without max subtraction
  (logits are bounded: |s/8| < 4), with block-causal skipping and a
  triangular mask multiply on diagonal 128-blocks only.
- A/V product accumulates O^T[d, q] with an extra ones-column in V giving the
  softmax row-sums in the same pass; normalization multiplies by the
  DMA-broadcast reciprocal row.
"""

import numpy as np
import ml_dtypes

import concourse.bass as bass
import concourse.mybir as mybir
import concourse.tile as tile
from concourse.bass_utils import run_bass_kernel_spmd

BATCH, CTX, ED = 2, 2048, 1024
NH, HD = 16, 64
ROPE_BASE = 10000.0
P = 128
NCORES = 8
HPC = 4  # heads per core
QS = 1024  # q supertile width
NSUP = CTX // QS  # 2
NKT = CTX // P  # 16 k-chunks / ctx tiles
NEC = ED // P  # 8 ED chunks
PT_BUFS = 6

F32 = mybir.dt.float32
BF16 = mybir.dt.bfloat16


def _split_multi_waits(nc, max_waits=1):
    """Walrus in this container rejects >1 sync wait per instruction; hoist
    extra waits onto preceding same-engine NoOps (semantically identical:
    engines execute their stream in order)."""
    n = 0
    for func in nc.m.functions:
        for bb in func.blocks:
            insts = list(bb.instructions)
            out = []
            changed = False
            for inst in insts:
                si = inst.sync_info
                if si and si.on_wait and len(si.on_wait) > max_waits:
                    waits = list(si.on_wait)
                    for k, w in enumerate(waits[:-max_waits]):
                        nop = mybir.InstNoOp(
                            name=f"{inst.name}-ws{k}",
                            sync_info=mybir.SyncInfo(on_wait=[w], on_update=[]),
                        )
                        nop.engine = inst.engine
                        out.append(nop)
                        n += 1
                    inst.sync_info = mybir.SyncInfo(
                        on_wait=waits[-max_waits:], on_update=list(si.on_update or [])
                    )
                    changed = True
                out.append(inst)
            if changed:
                bb.instructions = out
    return n


def _emit(nc, xT, wq, wk, wv, wo, sin2, tri, z, tc):
    import contextlib

    Exp = mybir.ActivationFunctionType.Exp
    Ln = mybir.ActivationFunctionType.Ln
    MULT = mybir.AluOpType.mult

    with contextlib.ExitStack() as ctx:
        pers = ctx.enter_context(tc.tile_pool(name="pers", bufs=1))
        work = ctx.enter_context(tc.tile_pool(name="work", bufs=4))
        ptpool = ctx.enter_context(tc.tile_pool(name="ptpool", bufs=PT_BUFS))
        psum = ctx.enter_context(tc.tile_pool(name="psum", bufs=2, space="PSUM"))
        dram = ctx.enter_context(tc.tile_pool(name="dram", bufs=4, space="DRAM"))

        xt_sb = pers.tile([P, NEC, CTX], BF16, tag="xt")
        wq_sb = pers.tile([P, NEC, 256], BF16, tag="wq")
        wk_sb = pers.tile([P, NEC, 256], BF16, tag="wk")
        wv_sb = pers.tile([P, NEC, 256], BF16, tag="wv")
        wo_sb = pers.tile([P, 2, ED], BF16, tag="wo")
        sin_sb = pers.tile([P, CTX], BF16, tag="sin")
        tri_sb = pers.tile([P, 512], BF16, tag="tri")
        qt_sb = pers.tile([P, 2, CTX], BF16, tag="qt")
        kt_sb = pers.tile([P, 2, CTX], BF16, tag="kt")
        v_sb = pers.tile([P, NKT, HPC, 66], BF16, tag="v")
        ot_sb = pers.tile([P, 2, CTX], BF16, tag="ot")

        # ---- loads: xt+wq+wk first (QK projections and scores start ASAP);
        # wv/wo deferred (V only needed once first exps exist) ----
        for c in range(NEC):
            nc.sync.dma_start(wq_sb[:, c, :], wq[c * P : (c + 1) * P, :])
            nc.sync.dma_start(wk_sb[:, c, :], wk[c * P : (c + 1) * P, :])
        nc.sync.dma_start(sin_sb[:], sin2)
        # x^T in 512-col pieces so projection matmuls chase arrivals at fine
        # grain (first exp ~10us earlier); j=0 needs pieces 0-1 first
        for piece in range(4):
            for c in range(NEC):
                nc.sync.dma_start(
                    xt_sb[:, c, piece * 512 : (piece + 1) * 512],
                    xT[c * P : (c + 1) * P, piece * 512 : (piece + 1) * 512],
                )
            if piece == 0:
                nc.sync.dma_start(tri_sb[:], tri)
        for c in range(NEC):
            nc.sync.dma_start(wv_sb[:, c, :], wv[c * P : (c + 1) * P, :])
        for cc in range(2):
            nc.sync.dma_start(wo_sb[:, cc, :], wo[cc * P : (cc + 1) * P, :])
        nc.vector.memset(v_sb[:, :, :, 64:65], 1.0)

        # ---- V projection (natural layout [ctx, head*64], per ctx tile) ----
        def v_proj(t):
            psv = psum.tile([P, 256], F32, tag="b")
            for c in range(NEC):
                nc.tensor.matmul(
                    psv[:],
                    lhsT=xt_sb[:, c, t * P : (t + 1) * P],
                    rhs=wv_sb[:, c, :],
                    start=(c == 0),
                    stop=(c == NEC - 1),
                )
            nc.any.tensor_copy(
                out=v_sb[:, t, :, 0:64],
                in_=psv[:].rearrange("p (h d) -> p h d", h=HPC),
            )

        # ---- Q or K projection + rope for one (pair, supertile) ----
        def qk_proj(p, j, which):
            w_sb, dst = (wq_sb, qt_sb) if which == "q" else (wk_sb, kt_sb)
            ps = psum.tile([P, QS], F32, tag="b")
            for c in range(NEC):
                for h2 in range(2):
                    nc.tensor.matmul(
                        ps[:, h2 * 512 : (h2 + 1) * 512],
                        lhsT=w_sb[:, c, p * P : (p + 1) * P],
                        rhs=xt_sb[:, c, j * QS + h2 * 512 : j * QS + (h2 + 1) * 512],
                        start=(c == 0),
                        stop=(c == NEC - 1),
                    )
            nc.vector.tensor_tensor(
                out=dst[:, p, j * QS : (j + 1) * QS],
                in0=ps[:],
                in1=sin_sb[:, j * QS : (j + 1) * QS],
                op=MULT,
            )

        # ---- out projection for one ctx tile (both pairs in PSUM) ----
        def out_proj(t):
            zp = psum.tile([P, ED], F32, tag="b")
            for cc in range(2):
                for nh in range(2):
                    nc.tensor.matmul(
                        zp[:, nh * 512 : (nh + 1) * 512],
                        lhsT=ot_sb[:, cc, t * P : (t + 1) * P],
                        rhs=wo_sb[:, cc, nh * 512 : (nh + 1) * 512],
                        start=(cc == 0),
                        stop=(cc == 1),
                    )
            zs = work.tile([P, ED], F32, tag="zs")
            nc.any.tensor_copy(out=zs[:], in_=zp[:])
            nc.sync.dma_start(z[t * P : (t + 1) * P, :], zs[:])

        # PE filler units (~1-3.5us each), pulled lazily between attention
        # chunks so the scheduler always has the exp-feeding scores first in
        # priority and dense PE work behind them.
        fillers = []

        def pull_filler():
            if fillers:
                fillers.pop(0)()

        # ---- attention for one (head, q-supertile) ----
        def attention(h, j):
            p, s = h // 2, h % 2
            qt_h = qt_sb[s * HD : (s + 1) * HD, p, :]
            kt_h = kt_sb[s * HD : (s + 1) * HD, p, :]
            ot = psum.tile([P, QS], F32, tag="b")
            nkt = (j + 1) * (QS // P)
            for kt in range(nkt):
                pull_filler()
                st = psum.tile([P, QS], F32, tag="a")
                d = kt * P - j * QS  # diag offset within supertile (<=0: full)
                for h2 in range(2):
                    if kt * P >= j * QS + (h2 + 1) * 512:
                        continue  # this q-half fully masked for this chunk
                    # clip to causally-live columns (cols < d are dead)
                    lo_rel = min(max(d - 512 * h2, 0), 512)
                    nc.tensor.matmul(
                        st[:, h2 * 512 + lo_rel : (h2 + 1) * 512],
                        lhsT=kt_h[:, kt * P : (kt + 1) * P],
                        rhs=qt_h[
                            :,
                            j * QS + h2 * 512 + lo_rel : j * QS + (h2 + 1) * 512,
                        ],
                        start=True,
                        stop=True,
                    )
                lo = max(d, 0)
                pt = ptpool.tile([P, QS], BF16, tag="pt")
                nc.scalar.activation(pt[:, lo:QS], st[:, lo:QS], Exp, scale=0.125)
                if d >= 0:
                    nc.vector.tensor_tensor(
                        out=pt[:, d : d + P],
                        in0=pt[:, d : d + P],
                        in1=tri_sb[:, 384:512],
                        op=MULT,
                    )
                for h2 in range(2):
                    if kt * P >= j * QS + (h2 + 1) * 512:
                        continue
                    lo_rel = min(max(d - 512 * h2, 0), 512)
                    last_kt = min(nkt, j * (QS // P) + 4 * (h2 + 1)) - 1
                    nc.tensor.matmul(
                        ot[0:65, h2 * 512 + lo_rel : (h2 + 1) * 512],
                        lhsT=v_sb[:, kt, h, 0:65],
                        rhs=pt[:, h2 * 512 + lo_rel : (h2 + 1) * 512],
                        start=(kt == 0),
                        stop=(kt == last_kt),
                    )
            # evict O^T+rowsum to SBUF immediately (frees the PSUM slot while
            # the reciprocal chain runs)
            ots = work.tile([65, QS], F32, tag="ots")
            nc.any.tensor_copy(out=ots[:], in_=ot[0:65, :])
            # 1/rowsum as exp(-ln(x)) on ACT (ACT Reciprocal is blocked,
            # custom-DVE recip ops don't compile under this walrus, DVE
            # reciprocal is 8 cyc/elem). The row is reshaped to [8, 128] by
            # DMA first so the two ACT ops run at FD=128 instead of 1024.
            rdraw = dram.tile([1, QS], F32, tag="rdraw")
            nc.sync.dma_start(rdraw[:], ots[64:65, :])
            rsplit = work.tile([8, QS // 8], F32, tag="rsplit")
            nc.sync.dma_start(
                rsplit[:], rdraw[0:1, :].rearrange("a (p c) -> (a p) c", p=8)
            )
            lrow = work.tile([8, QS // 8], F32, tag="lrow")
            nc.scalar.activation(lrow[:], rsplit[:], Ln)
            rrow = work.tile([8, QS // 8], F32, tag="rrow")
            nc.scalar.activation(rrow[:], lrow[:], Exp, scale=-1.0)
            rdram = dram.tile([1, QS], F32, tag="rdram")
            nc.sync.dma_start(
                rdram[0:1, :].rearrange("a (p c) -> (a p) c", p=8), rrow[:]
            )
            for nh in range(2):
                bcast = work.tile([64, 512], F32, tag="bcast")
                nc.sync.dma_start(
                    bcast[:],
                    rdram[0:1, nh * 512 : (nh + 1) * 512].to_broadcast((64, 512)),
                )
                nc.vector.tensor_tensor(
                    out=ot_sb[
                        s * HD : (s + 1) * HD,
                        p,
                        j * QS + nh * 512 : j * QS + (nh + 1) * 512,
                    ],
                    in0=ots[0:64, nh * 512 : (nh + 1) * 512],
                    in1=bcast[:],
                    op=MULT,
                )

        # ---- emission schedule ----
        # Fillers are pulled BEFORE each attention chunk; the queue order
        # guarantees every filler is emitted before its first consumer
        # (emission order IS Tile's program order).
        qk_proj(0, 0, "q")
        qk_proj(0, 0, "k")
        # attn(0,0) chunk kt pulls v_proj(kt) right before using v_sb[kt]
        fillers.extend([(lambda t=t: v_proj(t)) for t in range(8)])
        attention(0, 0)
        fillers.append(lambda: qk_proj(1, 0, "q"))
        fillers.append(lambda: qk_proj(1, 0, "k"))
        fillers.extend([(lambda t=t: v_proj(t)) for t in range(8, 14)])
        attention(1, 0)
        fillers.extend([(lambda t=t: v_proj(t)) for t in range(14, NKT)])
        fillers.append(lambda: qk_proj(0, 1, "q"))
        fillers.append(lambda: qk_proj(0, 1, "k"))
        fillers.append(lambda: qk_proj(1, 1, "q"))
        fillers.append(lambda: qk_proj(1, 1, "k"))
        attention(2, 0)
        attention(3, 0)
        while fillers:
            pull_filler()
        fillers.extend([(lambda t=t: out_proj(t)) for t in range(NKT // 2)])
        attention(0, 1)
        attention(1, 1)
        attention(2, 1)
        attention(3, 1)
        while fillers:
            pull_filler()
        for t in range(NKT // 2, NKT):
            out_proj(t)


def _build_program(split_waits=True):
    nc = bass.Bass("TRN2", target_bir_lowering=False, debug=False, num_devices=NCORES)
    xT = nc.dram_tensor("xT", [ED, CTX], BF16, kind="ExternalInput").ap()
    wq = nc.dram_tensor("wq", [ED, 256], BF16, kind="ExternalInput").ap()
    wk = nc.dram_tensor("wk", [ED, 256], BF16, kind="ExternalInput").ap()
    wv = nc.dram_tensor("wv", [ED, 256], BF16, kind="ExternalInput").ap()
    wo = nc.dram_tensor("wo", [256, ED], BF16, kind="ExternalInput").ap()
    sin2 = nc.dram_tensor("sin2", [P, CTX], BF16, kind="ExternalInput").ap()
    tri = nc.dram_tensor("tri", [P, 512], BF16, kind="ExternalInput").ap()
    z = nc.dram_tensor("z", [CTX, ED], F32, kind="ExternalOutput").ap()
    with tile.TileContext(nc) as tc:
        _emit(nc, xT, wq, wk, wv, wo, sin2, tri, z, tc)
    if split_waits:
        _split_multi_waits(nc)
    return nc


_PROGRAM = None


def _get_program():
    global _PROGRAM
    if _PROGRAM is None:
        _PROGRAM = _build_program()
    return _PROGRAM


def _host_tables():
    # rotate-half fold matrix: q_rot = R q
    Rm = np.zeros((HD, HD), np.float32)
    for i in range(HD // 2):
        Rm[i, i] = 1.0
        Rm[i, i + 32] = -1.0
        Rm[i + 32, i + 32] = 1.0
        Rm[i + 32, i] = 1.0
    j = np.arange(HD // 2, dtype=np.float32)
    thetas = 1.0 / ROPE_BASE ** (2.0 * j / (HD // 2))
    pos = np.arange(CTX, dtype=np.float32)
    ang = pos[:, None] * thetas[None, :]
    sinT = np.sin(np.concatenate([ang, ang], axis=-1)).T.astype(np.float32)  # [64,CTX]
    sin2 = np.ascontiguousarray(np.tile(sinT, (2, 1))).astype(
        ml_dtypes.bfloat16
    )  # [128, CTX]
    # combined mask [128, 512]: tri[i, c] = 1 iff c >= 384 + i; the slice
    # tri[:, 512-W:] gives zeros on the first W-128 cols and the causal
    # triangle on the last 128
    cg = np.arange(512)[None, :]
    ii = np.arange(P)[:, None]
    tri = (cg >= 384 + ii).astype(np.float32).astype(ml_dtypes.bfloat16)
    return Rm, sin2, tri


def _run(x, Wq, Wk, Wv, Wo):
    nc = _get_program()
    Rm, sin2, tri = _host_tables()

    def fold(W):
        W2 = W.reshape(ED, NH, HD)
        return np.einsum("enh,gh->eng", W2, Rm).reshape(ED, NH * HD)

    bf = ml_dtypes.bfloat16
    Wq_f = fold(Wq).astype(bf)
    Wk_f = fold(Wk).astype(bf)
    Wv_b = Wv.astype(bf)
    Wo_b = Wo.astype(bf)
    xT_b = [np.ascontiguousarray(x[b].T).astype(bf) for b in range(BATCH)]

    in_maps = []
    for core in range(NCORES):
        b, g = core // 4, core % 4
        cs = slice(256 * g, 256 * (g + 1))
        in_maps.append(
            {
                "xT": xT_b[b],
                "wq": np.ascontiguousarray(Wq_f[:, cs]),
                "wk": np.ascontiguousarray(Wk_f[:, cs]),
                "wv": np.ascontiguousarray(Wv_b[:, cs]),
                "wo": np.ascontiguousarray(Wo_b[cs, :]),
                "sin2": sin2,
                "tri": tri,
            }
        )
    return nc, in_maps


def kernel(x, Wq, Wk, Wv, Wo, bo):
    x = np.asarray(x, dtype=np.float32)
    nc, in_maps = _run(x, np.asarray(Wq, np.float32), np.asarray(Wk, np.float32),
                       np.asarray(Wv, np.float32), np.asarray(Wo, np.float32))
    res = run_bass_kernel_spmd(nc, in_maps, core_ids=list(range(NCORES)))
    out = np.zeros((BATCH, CTX, ED), np.float32)
    for core in range(NCORES):
        b = core // 4
        out[b] += res.results[core]["z"]
    out += np.asarray(bo, np.float32)[None, None, :]
    return out



# revision 5
# speedup vs baseline: 1.1988x; 1.1988x over previous
"""Causal multi-head attention with RoPE (faithful to reference's cos<-sin
overwrite bug) on 8 TRN2 NeuronCores.

Sharding: data parallel on batch (2) x tensor parallel on heads (4 groups of
4 heads) = 8 cores. Each core computes, for its (batch, 4 heads), flash-style
causal attention and its partial out-projection; the host sums the 4 partials
per batch and adds the bias.

Key structure (v2):
- RoPE's rotate-half is folded into Wq/Wk on the host; on-device rope is one
  elementwise multiply by a sin table.
- Scores are computed transposed (S^T[k, q]) per 128-row k-chunk over a
  1024-wide q supertile, exp'd without max subtraction (logits bounded), with
  a triangular mask multiply on the diagonal 128-block only.
- Optionally (USE_FP8) the rope multiply emits fp8e4 q/k, which are repacked
  by DMA into [32, 2, ctx] DoubleRow layout; score matmuls then run in
  MatmulPerfMode.DoubleRow at half cost.
- A/V runs in output-partition form: per 128-q tile, O[q, d] accumulates in
  PSUM over k-chunks with pt chunks as the stationary operand; an extra ones
  column in V yields softmax row-sums in the same pass. Normalization is a
  DVE reciprocal ([128,1]) + per-partition tensor_scalar multiply during the
  PSUM eviction.
- Normalized O tiles are pair-transposed on the PE (via identity) into
  [d, q] layout for the out-projection; z is staged in SBUF and written to
  DRAM two 128-row tiles per DMA.
"""

import contextlib

import numpy as np
import ml_dtypes

import concourse.bass as bass
import concourse.mybir as mybir
import concourse.tile as tile
from concourse.bass_utils import run_bass_kernel_spmd

BATCH, CTX, ED = 2, 2048, 1024
NH, HD = 16, 64
ROPE_BASE = 10000.0
P = 128
NCORES = 8
HPC = 4  # heads per core
QS = 1024  # q supertile width
NKT = CTX // P  # 16 k-chunks
NEC = ED // P  # 8 contraction chunks

F32 = mybir.dt.float32
BF16 = mybir.dt.bfloat16
FP8 = mybir.dt.float8e4

USE_FP8 = False  # fp8e4 DoubleRow score matmuls


def _split_multi_waits(nc, max_waits=1):
    """Walrus in this container rejects >1 sync wait per instruction; hoist
    extra waits onto preceding same-engine NoOps (semantically identical:
    engines execute their stream in order)."""
    n = 0
    for func in nc.m.functions:
        for bb in func.blocks:
            insts = list(bb.instructions)
            out = []
            changed = False
            for inst in insts:
                si = inst.sync_info
                if si and si.on_wait and len(si.on_wait) > max_waits:
                    waits = list(si.on_wait)
                    for k, w in enumerate(waits[:-max_waits]):
                        nop = mybir.InstNoOp(
                            name=f"{inst.name}-ws{k}",
                            sync_info=mybir.SyncInfo(on_wait=[w], on_update=[]),
                        )
                        nop.engine = inst.engine
                        out.append(nop)
                        n += 1
                    inst.sync_info = mybir.SyncInfo(
                        on_wait=waits[-max_waits:], on_update=list(si.on_update or [])
                    )
                    changed = True
                out.append(inst)
            if changed:
                bb.instructions = out
    return n


def _emit(nc, xT, wq, wk, wv, wo, sin2, tri, ident, z, tc):
    Exp = mybir.ActivationFunctionType.Exp
    Copy = mybir.ActivationFunctionType.Copy
    MULT = mybir.AluOpType.mult
    DR = mybir.MatmulPerfMode.DoubleRow

    with contextlib.ExitStack() as ctx:
        pers = ctx.enter_context(tc.tile_pool(name="pers", bufs=1))
        ptp = ctx.enter_context(tc.tile_pool(name="ptp", bufs=20))
        work = ctx.enter_context(tc.tile_pool(name="work", bufs=2))
        psum = ctx.enter_context(tc.tile_pool(name="psum", bufs=1, space="PSUM"))

        xt_sb = pers.tile([P, NEC, CTX], BF16, tag="xt")
        wq_sb = pers.tile([P, NEC, 256], BF16, tag="wq")
        wk_sb = pers.tile([P, NEC, 256], BF16, tag="wk")
        wv_sb = pers.tile([P, NEC, 256], BF16, tag="wv")
        wo_sb = pers.tile([P, 2, ED], BF16, tag="wo")
        sin_sb = pers.tile([P, CTX], BF16, tag="sin")
        tri_sb = pers.tile([P, P], BF16, tag="tri")
        id_sb = pers.tile([P, P], BF16, tag="id")
        v_sb = pers.tile([P, NKT, HPC, 66], BF16, tag="v")
        ot_sb = pers.tile([P, 2, CTX], BF16, tag="ot")
        if USE_FP8:
            q8p = pers.tile([P, 2, CTX], FP8, tag="q8p")
            k8p = pers.tile([P, 2, CTX], FP8, tag="k8p")
            q8 = pers.tile([32, HPC, 2, CTX], FP8, tag="q8")
            k8 = pers.tile([32, HPC, 2, CTX], FP8, tag="k8")
        else:
            qt_sb = pers.tile([P, 2, CTX], BF16, tag="qt")
            kt_sb = pers.tile([P, 2, CTX], BF16, tag="kt")

        # ---- loads: fine-grained at the front so projections start ASAP ----
        def ld(dst, src):
            nc.sync.dma_start(dst, src)

        ld(wq_sb[:, 0:4, :], wq[0:512, :].rearrange("(c p) n -> p c n", p=P))
        ld(xt_sb[:, 0:4, 0:512], xT[0:512, 0:512].rearrange("(c p) n -> p c n", p=P))
        ld(wq_sb[:, 4:8, :], wq[512:1024, :].rearrange("(c p) n -> p c n", p=P))
        ld(xt_sb[:, 4:8, 0:512], xT[512:1024, 0:512].rearrange("(c p) n -> p c n", p=P))
        ld(wk_sb[:, 0:4, :], wk[0:512, :].rearrange("(c p) n -> p c n", p=P))
        ld(wk_sb[:, 4:8, :], wk[512:1024, :].rearrange("(c p) n -> p c n", p=P))
        ld(sin_sb[:], sin2)
        ld(
            xt_sb[:, :, 512:1024],
            xT[:, 512:1024].rearrange("(c p) n -> p c n", p=P),
        )
        ld(tri_sb[:], tri)
        for c2 in range(2):
            ld(
                wv_sb[:, 4 * c2 : 4 * c2 + 4, :],
                wv[512 * c2 : 512 * (c2 + 1), :].rearrange("(c p) n -> p c n", p=P),
            )
        ld(
            xt_sb[:, :, 1024:1536],
            xT[:, 1024:1536].rearrange("(c p) n -> p c n", p=P),
        )
        ld(id_sb[:], ident)
        ld(
            xt_sb[:, :, 1536:2048],
            xT[:, 1536:2048].rearrange("(c p) n -> p c n", p=P),
        )
        ld(wo_sb[:], wo.rearrange("(cc p) n -> p cc n", p=P))
        nc.gpsimd.memset(v_sb[:, :, :, 64:65], 1.0)

        # ---- Q/K projection + rope for one (which, pair, supertile, half) ----
        def qk_proj(which, p, j, half):
            w_sb = wq_sb if which == "q" else wk_sb
            ps = psum.tile([P, ED], F32, tag="aux", bufs=1)
            c0 = j * QS + half * 512
            for c in range(NEC):
                nc.tensor.matmul(
                    ps[:, 0:512],
                    lhsT=w_sb[:, c, p * P : (p + 1) * P],
                    rhs=xt_sb[:, c, c0 : c0 + 512],
                    start=(c == 0),
                    stop=(c == NEC - 1),
                )
            if USE_FP8:
                dst = q8p if which == "q" else k8p
            else:
                dst = qt_sb if which == "q" else kt_sb
            nc.vector.tensor_tensor(
                out=dst[:, p, c0 : c0 + 512],
                in0=ps[:, 0:512],
                in1=sin_sb[:, c0 : c0 + 512],
                op=MULT,
            )
            if USE_FP8 and half == 1:
                src, pk = (q8p, q8) if which == "q" else (k8p, k8)
                for s in range(2):
                    for jj in range(2):
                        ld(
                            pk[:, 2 * p + s, jj, j * QS : (j + 1) * QS],
                            src[
                                s * HD + 32 * jj : s * HD + 32 * jj + 32,
                                p,
                                j * QS : (j + 1) * QS,
                            ],
                        )

        # ---- V projection (natural layout, per ctx tile) ----
        def v_proj(t):
            ps = psum.tile([P, 256], F32, tag="vp", bufs=1)
            for c in range(NEC):
                nc.tensor.matmul(
                    ps[:],
                    lhsT=xt_sb[:, c, t * P : (t + 1) * P],
                    rhs=wv_sb[:, c, :],
                    start=(c == 0),
                    stop=(c == NEC - 1),
                )
            nc.vector.tensor_copy(
                out=v_sb[:, t, :, 0:64],
                in_=ps[:].rearrange("p (h d) -> p h d", h=HPC),
            )

        # ---- scores+exp(+mask) for one (head, supertile, k-chunk) ----
        def scores(h, j, KT):
            p, s = h // 2, h % 2
            st = psum.tile([P, QS], F32, tag="st", bufs=2)
            lo = max(KT * P - j * QS, 0)
            for half in (0, 1):
                a = max(lo, half * 512)
                b = (half + 1) * 512
                if a >= b:
                    continue
                if USE_FP8:
                    nc.tensor.matmul(
                        st[:, a:b],
                        lhsT=k8[:, h, :, KT * P : (KT + 1) * P],
                        rhs=q8[:, h, :, j * QS + a : j * QS + b],
                        start=True,
                        stop=True,
                        perf_mode=DR,
                    )
                else:
                    nc.tensor.matmul(
                        st[:, a:b],
                        lhsT=kt_sb[s * HD : (s + 1) * HD, p, KT * P : (KT + 1) * P],
                        rhs=qt_sb[s * HD : (s + 1) * HD, p, j * QS + a : j * QS + b],
                        start=True,
                        stop=True,
                    )
            pt = ptp.tile([P, QS], BF16, tag="pt")
            nc.scalar.activation(pt[:, lo:QS], st[:, lo:QS], Exp, scale=0.125)
            if KT >= 8 * j:
                d = KT * P - j * QS
                nc.vector.tensor_tensor(
                    out=pt[:, d : d + P], in0=pt[:, d : d + P], in1=tri_sb[:], op=MULT
                )
            return pt, lo

        # ---- A/V for one (head, supertile, local q tile) in O[q,d] form ----
        def av(h, j, t, pts, osb_t):
            T = 8 * j + t
            o = psum.tile([P, 66], F32, tag="o", bufs=1)
            for KT in range(T + 1):
                pt, lo = pts[KT]
                off = t * P  # pt cols are q-local; cols [0:lo) never read
                nc.tensor.matmul(
                    o[:, 0:65],
                    lhsT=pt[:, off : off + P],
                    rhs=v_sb[:, KT, h, 0:65],
                    start=(KT == 0),
                    stop=(KT == T),
                )
            rc = work.tile([P, 1], F32, tag="rc", bufs=6)
            nc.vector.reciprocal(rc[:], o[:, 64:65])
            nc.vector.tensor_scalar(
                out=osb_t[:, h, :],
                in0=o[:, 0:64],
                scalar1=rc[:],
                scalar2=None,
                op0=MULT,
            )

        # ---- pair transpose of normalized O into [d, q] for out_proj ----
        def transpose_pair(cc, T, osb_t):
            tr = psum.tile([P, P], BF16, tag="aux", bufs=1)
            nc.tensor.transpose(tr[:], osb_t[:, 2 * cc : 2 * cc + 2, :], id_sb[:])
            nc.vector.tensor_copy(out=ot_sb[:, cc, T * P : (T + 1) * P], in_=tr[:])

        # ---- out projection for one ctx tile; z staged 2 tiles per DMA ----
        zstage = {}

        def out_proj(T):
            zp = psum.tile([P, ED], F32, tag="aux", bufs=1)
            for cc in (0, 1):
                for nh in (0, 1):
                    nc.tensor.matmul(
                        zp[:, nh * 512 : (nh + 1) * 512],
                        lhsT=ot_sb[:, cc, T * P : (T + 1) * P],
                        rhs=wo_sb[:, cc, nh * 512 : (nh + 1) * 512],
                        start=(cc == 0),
                        stop=(cc == 1),
                    )
            if T % 2 == 0:
                zstage[T // 2] = work.tile([P, 2, ED], F32, tag="zs", bufs=2, name=f"zs{T}")
            zs_t = zstage[T // 2]
            if T % 2 == 0:
                nc.vector.tensor_copy(out=zs_t[:, 0, :], in_=zp[:])
            else:
                nc.scalar.activation(zs_t[:, 1, :], zp[:], Copy)
                ld(
                    z[(T - 1) * P : (T + 1) * P, :].rearrange(
                        "(a p) n -> p a n", p=P
                    ),
                    zs_t[:],
                )

        # ---- emission schedule ----
        # Fillers are PE work units pulled between attention chunks so the
        # exp-feeding scores stay first in priority with dense PE work behind.
        fillers = []

        def pull_filler():
            if fillers:
                fillers.pop(0)()

        for p, half in ((0, 0), (0, 1)):
            qk_proj("q", p, 0, half)
            qk_proj("k", p, 0, half)

        osb = {}

        def run_supertile(j):
            nkt = 8 * (j + 1)
            for h in range(HPC):
                pts = {}
                for KT in range(nkt):
                    pull_filler()
                    pts[KT] = scores(h, j, KT)
                    t = KT - 8 * j
                    if t >= 0:
                        T = KT
                        if h == 0:
                            osb[T] = work.tile([P, HPC, 64], BF16, tag="osb", bufs=16, name=f"osb{T}")
                        av(h, j, t, pts, osb[T])
                        if h == 1:
                            fillers.append(
                                lambda T=T: transpose_pair(0, T, osb[T])
                            )
                        if h == 3:
                            transpose_pair(1, T, osb[T])
                            out_proj(T)

        # fillers consumed during supertile 0 (32 chunks): remaining j0
        # projections, all 16 v tiles, then the j1 q/k projections
        fillers.extend([(lambda t=t: v_proj(t)) for t in range(4)])
        for p, half in ((1, 0), (1, 1)):
            fillers.append(lambda p=p, half=half: qk_proj("q", p, 0, half))
            fillers.append(lambda p=p, half=half: qk_proj("k", p, 0, half))
        fillers.extend([(lambda t=t: v_proj(t)) for t in range(4, 16)])
        for p in (0, 1):
            for half in (0, 1):
                fillers.append(lambda p=p, half=half: qk_proj("q", p, 1, half))
                fillers.append(lambda p=p, half=half: qk_proj("k", p, 1, half))

        run_supertile(0)
        run_supertile(1)
        while fillers:
            pull_filler()


def _build_program(split_waits=True):
    nc = bass.Bass("TRN2", target_bir_lowering=False, debug=False, num_devices=NCORES)
    xT = nc.dram_tensor("xT", [ED, CTX], BF16, kind="ExternalInput").ap()
    wq = nc.dram_tensor("wq", [ED, 256], BF16, kind="ExternalInput").ap()
    wk = nc.dram_tensor("wk", [ED, 256], BF16, kind="ExternalInput").ap()
    wv = nc.dram_tensor("wv", [ED, 256], BF16, kind="ExternalInput").ap()
    wo = nc.dram_tensor("wo", [256, ED], BF16, kind="ExternalInput").ap()
    sin2 = nc.dram_tensor("sin2", [P, CTX], BF16, kind="ExternalInput").ap()
    tri = nc.dram_tensor("tri", [P, P], BF16, kind="ExternalInput").ap()
    ident = nc.dram_tensor("ident", [P, P], BF16, kind="ExternalInput").ap()
    z = nc.dram_tensor("z", [CTX, ED], F32, kind="ExternalOutput").ap()
    with tile.TileContext(nc) as tc:
        _emit(nc, xT, wq, wk, wv, wo, sin2, tri, ident, z, tc)
    if split_waits:
        _split_multi_waits(nc)
    return nc


_PROGRAM = None


def _get_program():
    global _PROGRAM
    if _PROGRAM is None:
        _PROGRAM = _build_program()
    return _PROGRAM


def _host_tables():
    # rotate-half fold matrix: q_rot = R q
    Rm = np.zeros((HD, HD), np.float32)
    for i in range(HD // 2):
        Rm[i, i] = 1.0
        Rm[i, i + 32] = -1.0
        Rm[i + 32, i + 32] = 1.0
        Rm[i + 32, i] = 1.0
    j = np.arange(HD // 2, dtype=np.float32)
    thetas = 1.0 / ROPE_BASE ** (2.0 * j / (HD // 2))
    pos = np.arange(CTX, dtype=np.float32)
    ang = pos[:, None] * thetas[None, :]
    sinT = np.sin(np.concatenate([ang, ang], axis=-1)).T.astype(np.float32)  # [64,CTX]
    sin2 = np.ascontiguousarray(np.tile(sinT, (2, 1))).astype(
        ml_dtypes.bfloat16
    )  # [128, CTX]
    cg = np.arange(P)[None, :]
    ii = np.arange(P)[:, None]
    tri = (cg >= ii).astype(np.float32).astype(ml_dtypes.bfloat16)  # keep q >= k
    ident = np.eye(P, dtype=np.float32).astype(ml_dtypes.bfloat16)
    return Rm, sin2, tri, ident


def _run(x, Wq, Wk, Wv, Wo):
    nc = _get_program()
    Rm, sin2, tri, ident = _host_tables()

    def fold(W):
        W2 = W.reshape(ED, NH, HD)
        return np.einsum("enh,gh->eng", W2, Rm).reshape(ED, NH * HD)

    bf = ml_dtypes.bfloat16
    Wq_f = fold(Wq).astype(bf)
    Wk_f = fold(Wk).astype(bf)
    Wv_b = Wv.astype(bf)
    Wo_b = Wo.astype(bf)
    xT_b = [np.ascontiguousarray(x[b].T).astype(bf) for b in range(BATCH)]

    in_maps = []
    for core in range(NCORES):
        b, g = core // 4, core % 4
        cs = slice(256 * g, 256 * (g + 1))
        in_maps.append(
            {
                "xT": xT_b[b],
                "wq": np.ascontiguousarray(Wq_f[:, cs]),
                "wk": np.ascontiguousarray(Wk_f[:, cs]),
                "wv": np.ascontiguousarray(Wv_b[:, cs]),
                "wo": np.ascontiguousarray(Wo_b[cs, :]),
                "sin2": sin2,
                "tri": tri,
                "ident": ident,
            }
        )
    return nc, in_maps


def kernel(x, Wq, Wk, Wv, Wo, bo):
    x = np.asarray(x, dtype=np.float32)
    nc, in_maps = _run(x, np.asarray(Wq, np.float32), np.asarray(Wk, np.float32),
                       np.asarray(Wv, np.float32), np.asarray(Wo, np.float32))
    res = run_bass_kernel_spmd(nc, in_maps, core_ids=list(range(NCORES)))
    out = np.zeros((BATCH, CTX, ED), np.float32)
    for core in range(NCORES):
        b = core // 4
        out[b] += res.results[core]["z"]
    out += np.asarray(bo, np.float32)[None, None, :]
    return out


# revision 6
# speedup vs baseline: 1.2535x; 1.0456x over previous
"""Causal multi-head attention with RoPE (faithful to reference's cos<-sin
overwrite bug) on 8 TRN2 NeuronCores.

Sharding: data parallel on batch (2) x tensor parallel on heads (4 groups of
4 heads) = 8 cores. Each core computes, for its (batch, 4 heads), flash-style
causal attention and its partial out-projection; the host sums the 4 partials
per batch and adds the bias.

Key structure (v2):
- RoPE's rotate-half is folded into Wq/Wk on the host; on-device rope is one
  elementwise multiply by a sin table.
- Scores are computed transposed (S^T[k, q]) per 128-row k-chunk over a
  1024-wide q supertile, exp'd without max subtraction (logits bounded), with
  a triangular mask multiply on the diagonal 128-block only.
- Optionally (USE_FP8) the rope multiply emits fp8e4 q/k, which are repacked
  by DMA into [32, 2, ctx] DoubleRow layout; score matmuls then run in
  MatmulPerfMode.DoubleRow at half cost.
- A/V runs in output-partition form: per 128-q tile, O[q, d] accumulates in
  PSUM over k-chunks with pt chunks as the stationary operand; an extra ones
  column in V yields softmax row-sums in the same pass. Normalization is a
  DVE reciprocal ([128,1]) + per-partition tensor_scalar multiply during the
  PSUM eviction.
- Normalized O tiles are pair-transposed on the PE (via identity) into
  [d, q] layout for the out-projection; z is staged in SBUF and written to
  DRAM two 128-row tiles per DMA.
"""

import contextlib

import numpy as np
import ml_dtypes

import concourse.bass as bass
import concourse.mybir as mybir
import concourse.tile as tile
from concourse.bass_utils import run_bass_kernel_spmd

BATCH, CTX, ED = 2, 2048, 1024
NH, HD = 16, 64
ROPE_BASE = 10000.0
P = 128
NCORES = 8
HPC = 4  # heads per core
QS = 1024  # q supertile width
NKT = CTX // P  # 16 k-chunks
NEC = ED // P  # 8 contraction chunks

F32 = mybir.dt.float32
BF16 = mybir.dt.bfloat16
FP8 = mybir.dt.float8e4

USE_FP8 = True  # fp8e4 DoubleRow score matmuls


def _split_multi_waits(nc, max_waits=1):
    """Walrus in this container rejects >1 sync wait per instruction; hoist
    extra waits onto preceding same-engine NoOps (semantically identical:
    engines execute their stream in order)."""
    n = 0
    for func in nc.m.functions:
        for bb in func.blocks:
            insts = list(bb.instructions)
            out = []
            changed = False
            for inst in insts:
                si = inst.sync_info
                if si and si.on_wait and len(si.on_wait) > max_waits:
                    waits = list(si.on_wait)
                    for k, w in enumerate(waits[:-max_waits]):
                        nop = mybir.InstNoOp(
                            name=f"{inst.name}-ws{k}",
                            sync_info=mybir.SyncInfo(on_wait=[w], on_update=[]),
                        )
                        nop.engine = inst.engine
                        out.append(nop)
                        n += 1
                    inst.sync_info = mybir.SyncInfo(
                        on_wait=waits[-max_waits:], on_update=list(si.on_update or [])
                    )
                    changed = True
                out.append(inst)
            if changed:
                bb.instructions = out
    return n


def _emit(nc, xT, wq, wk, wv, wo, sin2, tri, ident, z, tc):
    Exp = mybir.ActivationFunctionType.Exp
    Copy = mybir.ActivationFunctionType.Copy
    MULT = mybir.AluOpType.mult
    DR = mybir.MatmulPerfMode.DoubleRow

    with contextlib.ExitStack() as ctx:
        pers = ctx.enter_context(tc.tile_pool(name="pers", bufs=1))
        ptp = ctx.enter_context(tc.tile_pool(name="ptp", bufs=20))
        work = ctx.enter_context(tc.tile_pool(name="work", bufs=2))
        psum = ctx.enter_context(tc.tile_pool(name="psum", bufs=1, space="PSUM"))

        xt_sb = pers.tile([P, NEC, CTX], BF16, tag="xt")
        wq_sb = pers.tile([P, NEC, 256], BF16, tag="wq")
        wk_sb = pers.tile([P, NEC, 256], BF16, tag="wk")
        wv_sb = pers.tile([P, NEC, 256], BF16, tag="wv")
        wo_sb = pers.tile([P, 2, ED], BF16, tag="wo")
        sin_sb = pers.tile([P, CTX], BF16, tag="sin")
        tri_sb = pers.tile([P, P], BF16, tag="tri")
        id_sb = pers.tile([P, P], BF16, tag="id")
        v_sb = pers.tile([P, NKT, HPC, 66], BF16, tag="v")
        ot_sb = pers.tile([P, 2, CTX], BF16, tag="ot")
        if USE_FP8:
            q8p = pers.tile([P, 2, CTX], FP8, tag="q8p")
            k8p = pers.tile([P, 2, CTX], FP8, tag="k8p")
            q8 = pers.tile([32, HPC, 2, CTX], FP8, tag="q8")
            k8 = pers.tile([32, HPC, 2, CTX], FP8, tag="k8")
        else:
            qt_sb = pers.tile([P, 2, CTX], BF16, tag="qt")
            kt_sb = pers.tile([P, 2, CTX], BF16, tag="kt")

        # ---- loads: fine-grained at the front so projections start ASAP ----
        def ld(dst, src):
            nc.sync.dma_start(dst, src)

        ld(wq_sb[:, 0:4, :], wq[0:512, :].rearrange("(c p) n -> p c n", p=P))
        ld(xt_sb[:, 0:4, 0:512], xT[0:512, 0:512].rearrange("(c p) n -> p c n", p=P))
        ld(sin_sb[:], sin2)
        ld(wq_sb[:, 4:8, :], wq[512:1024, :].rearrange("(c p) n -> p c n", p=P))
        ld(xt_sb[:, 4:8, 0:512], xT[512:1024, 0:512].rearrange("(c p) n -> p c n", p=P))
        ld(wk_sb[:, 0:4, :], wk[0:512, :].rearrange("(c p) n -> p c n", p=P))
        ld(wk_sb[:, 4:8, :], wk[512:1024, :].rearrange("(c p) n -> p c n", p=P))
        ld(
            xt_sb[:, 0:4, 512:1024],
            xT[0:512, 512:1024].rearrange("(c p) n -> p c n", p=P),
        )
        ld(
            xt_sb[:, 4:8, 512:1024],
            xT[512:1024, 512:1024].rearrange("(c p) n -> p c n", p=P),
        )
        ld(tri_sb[:], tri)
        for c2 in range(2):
            ld(
                wv_sb[:, 4 * c2 : 4 * c2 + 4, :],
                wv[512 * c2 : 512 * (c2 + 1), :].rearrange("(c p) n -> p c n", p=P),
            )
        ld(
            xt_sb[:, :, 1024:1536],
            xT[:, 1024:1536].rearrange("(c p) n -> p c n", p=P),
        )
        ld(id_sb[:], ident)
        ld(
            xt_sb[:, :, 1536:2048],
            xT[:, 1536:2048].rearrange("(c p) n -> p c n", p=P),
        )
        ld(wo_sb[:], wo.rearrange("(cc p) n -> p cc n", p=P))
        nc.gpsimd.memset(v_sb[:, :, :, 64:65], 1.0)

        # ---- Q/K projection + rope for one (which, pair, supertile, half) ----
        def qk_proj(which, p, j, half, ptag="aux"):
            w_sb = wq_sb if which == "q" else wk_sb
            if ptag == "st":
                ps = psum.tile([P, QS], F32, tag="st", bufs=2, name="psqk")
            else:
                ps = psum.tile([P, ED], F32, tag="aux", bufs=1, name="psqk")
            c0 = j * QS + half * 512
            for c in range(NEC):
                nc.tensor.matmul(
                    ps[:, 0:512],
                    lhsT=w_sb[:, c, p * P : (p + 1) * P],
                    rhs=xt_sb[:, c, c0 : c0 + 512],
                    start=(c == 0),
                    stop=(c == NEC - 1),
                )
            if USE_FP8:
                dst = q8p if which == "q" else k8p
            else:
                dst = qt_sb if which == "q" else kt_sb
            nc.vector.tensor_tensor(
                out=dst[:, p, c0 : c0 + 512],
                in0=ps[:, 0:512],
                in1=sin_sb[:, c0 : c0 + 512],
                op=MULT,
            )
            if USE_FP8 and half == 1:
                src, pk = (q8p, q8) if which == "q" else (k8p, k8)
                for s in range(2):
                    for jj in range(2):
                        ld(
                            pk[:, 2 * p + s, jj, j * QS : (j + 1) * QS],
                            src[
                                s * HD + 32 * jj : s * HD + 32 * jj + 32,
                                p,
                                j * QS : (j + 1) * QS,
                            ],
                        )

        # ---- V projection (natural layout, per ctx tile) ----
        def v_proj(t):
            ps = psum.tile([P, 256], F32, tag="small", bufs=2)
            for c in range(NEC):
                nc.tensor.matmul(
                    ps[:],
                    lhsT=xt_sb[:, c, t * P : (t + 1) * P],
                    rhs=wv_sb[:, c, :],
                    start=(c == 0),
                    stop=(c == NEC - 1),
                )
            nc.vector.tensor_copy(
                out=v_sb[:, t, :, 0:64],
                in_=ps[:].rearrange("p (h d) -> p h d", h=HPC),
            )

        # ---- scores+exp(+mask) for one (head, supertile, k-chunk) ----
        def scores(h, j, KT):
            p, s = h // 2, h % 2
            st = psum.tile([P, QS], F32, tag="st", bufs=2)
            lo = max(KT * P - j * QS, 0)
            for half in (0, 1):
                a = max(lo, half * 512)
                b = (half + 1) * 512
                if a >= b:
                    continue
                if USE_FP8:
                    nc.tensor.matmul(
                        st[:, a:b],
                        lhsT=k8[:, h, :, KT * P : (KT + 1) * P],
                        rhs=q8[:, h, :, j * QS + a : j * QS + b],
                        start=True,
                        stop=True,
                        perf_mode=DR,
                    )
                else:
                    nc.tensor.matmul(
                        st[:, a:b],
                        lhsT=kt_sb[s * HD : (s + 1) * HD, p, KT * P : (KT + 1) * P],
                        rhs=qt_sb[s * HD : (s + 1) * HD, p, j * QS + a : j * QS + b],
                        start=True,
                        stop=True,
                    )
            pt = ptp.tile([P, QS], BF16, tag="pt")
            nc.scalar.activation(pt[:, lo:QS], st[:, lo:QS], Exp, scale=0.125)
            if KT >= 8 * j:
                d = KT * P - j * QS
                nc.vector.tensor_tensor(
                    out=pt[:, d : d + P], in0=pt[:, d : d + P], in1=tri_sb[:], op=MULT
                )
            return pt, lo

        # ---- A/V for one (head, supertile, local q tile) in O[q,d] form ----
        def av(h, j, t, pts, osb_t):
            T = 8 * j + t
            o = psum.tile([P, 66], F32, tag="small", bufs=2)
            for KT in range(T + 1):
                pt, lo = pts[KT]
                off = t * P  # pt cols are q-local; cols [0:lo) never read
                nc.tensor.matmul(
                    o[:, 0:65],
                    lhsT=pt[:, off : off + P],
                    rhs=v_sb[:, KT, h, 0:65],
                    start=(KT == 0),
                    stop=(KT == T),
                )
            rc = work.tile([P, 1], F32, tag="rc", bufs=6)
            nc.vector.reciprocal(rc[:], o[:, 64:65])
            nc.vector.tensor_scalar(
                out=osb_t[:, h, :],
                in0=o[:, 0:64],
                scalar1=rc[:],
                scalar2=None,
                op0=MULT,
            )

        # ---- pair transpose of normalized O into [d, q] for out_proj ----
        def transpose_pair(cc, T, osb_t):
            tr = psum.tile([P, P], BF16, tag="small", bufs=2)
            nc.tensor.transpose(tr[:], osb_t[:, 2 * cc : 2 * cc + 2, :], id_sb[:])
            nc.vector.tensor_copy(out=ot_sb[:, cc, T * P : (T + 1) * P], in_=tr[:])

        # ---- out projection for one ctx tile; z staged 2 tiles per DMA ----
        zstage = {}

        def out_proj(T):
            zp = psum.tile([P, ED], F32, tag="aux", bufs=1)
            for cc in (0, 1):
                for nh in (0, 1):
                    nc.tensor.matmul(
                        zp[:, nh * 512 : (nh + 1) * 512],
                        lhsT=ot_sb[:, cc, T * P : (T + 1) * P],
                        rhs=wo_sb[:, cc, nh * 512 : (nh + 1) * 512],
                        start=(cc == 0),
                        stop=(cc == 1),
                    )
            if T % 2 == 0:
                zstage[T // 2] = work.tile([P, 2, ED], F32, tag="zs", bufs=2, name=f"zs{T}")
            zs_t = zstage[T // 2]
            if T % 2 == 0:
                nc.vector.tensor_copy(out=zs_t[:, 0, :], in_=zp[:])
            else:
                nc.scalar.activation(zs_t[:, 1, :], zp[:], Copy)
                ld(
                    z[(T - 1) * P : (T + 1) * P, :].rearrange(
                        "(a p) n -> p a n", p=P
                    ),
                    zs_t[:],
                )

        # ---- emission schedule ----
        # Fillers are PE work units pulled between attention chunks so the
        # exp-feeding scores stay first in priority with dense PE work behind.
        fillers = []

        def pull_filler():
            if fillers:
                fillers.pop(0)()

        for p, half in ((0, 0), (0, 1)):
            qk_proj("q", p, 0, half, ptag="st")
            qk_proj("k", p, 0, half, ptag="st")

        osb = {}

        def run_supertile(j):
            nkt = 8 * (j + 1)
            for h in range(HPC):
                pts = {}
                for KT in range(nkt):
                    pull_filler()
                    pts[KT] = scores(h, j, KT)
                    t = KT - 8 * j
                    if t >= 0:
                        T = KT
                        if h == 0:
                            osb[T] = work.tile([P, HPC, 64], BF16, tag="osb", bufs=16, name=f"osb{T}")
                        av(h, j, t, pts, osb[T])
                        if h == 1:
                            fillers.append(
                                lambda T=T: transpose_pair(0, T, osb[T])
                            )
                        if h == 3:
                            transpose_pair(1, T, osb[T])
                            out_proj(T)

        # fillers consumed during supertile 0 (32 chunks): remaining j0
        # projections, all 16 v tiles, then the j1 q/k projections
        fillers.extend([(lambda t=t: v_proj(t)) for t in range(4)])
        for p, half in ((1, 0), (1, 1)):
            fillers.append(lambda p=p, half=half: qk_proj("q", p, 0, half))
            fillers.append(lambda p=p, half=half: qk_proj("k", p, 0, half))
        fillers.extend([(lambda t=t: v_proj(t)) for t in range(4, 16)])
        for p in (0, 1):
            for half in (0, 1):
                fillers.append(lambda p=p, half=half: qk_proj("q", p, 1, half))
                fillers.append(lambda p=p, half=half: qk_proj("k", p, 1, half))

        run_supertile(0)
        run_supertile(1)
        while fillers:
            pull_filler()


def _build_program(split_waits=True):
    nc = bass.Bass("TRN2", target_bir_lowering=False, debug=False, num_devices=NCORES)
    xT = nc.dram_tensor("xT", [ED, CTX], BF16, kind="ExternalInput").ap()
    wq = nc.dram_tensor("wq", [ED, 256], BF16, kind="ExternalInput").ap()
    wk = nc.dram_tensor("wk", [ED, 256], BF16, kind="ExternalInput").ap()
    wv = nc.dram_tensor("wv", [ED, 256], BF16, kind="ExternalInput").ap()
    wo = nc.dram_tensor("wo", [256, ED], BF16, kind="ExternalInput").ap()
    sin2 = nc.dram_tensor("sin2", [P, CTX], BF16, kind="ExternalInput").ap()
    tri = nc.dram_tensor("tri", [P, P], BF16, kind="ExternalInput").ap()
    ident = nc.dram_tensor("ident", [P, P], BF16, kind="ExternalInput").ap()
    z = nc.dram_tensor("z", [CTX, ED], F32, kind="ExternalOutput").ap()
    with tile.TileContext(nc) as tc:
        _emit(nc, xT, wq, wk, wv, wo, sin2, tri, ident, z, tc)
    if split_waits:
        _split_multi_waits(nc)
    return nc


_PROGRAM = None


def _get_program():
    global _PROGRAM
    if _PROGRAM is None:
        _PROGRAM = _build_program()
    return _PROGRAM


def _host_tables():
    # rotate-half fold matrix: q_rot = R q
    Rm = np.zeros((HD, HD), np.float32)
    for i in range(HD // 2):
        Rm[i, i] = 1.0
        Rm[i, i + 32] = -1.0
        Rm[i + 32, i + 32] = 1.0
        Rm[i + 32, i] = 1.0
    j = np.arange(HD // 2, dtype=np.float32)
    thetas = 1.0 / ROPE_BASE ** (2.0 * j / (HD // 2))
    pos = np.arange(CTX, dtype=np.float32)
    ang = pos[:, None] * thetas[None, :]
    sinT = np.sin(np.concatenate([ang, ang], axis=-1)).T.astype(np.float32)  # [64,CTX]
    sin2 = np.ascontiguousarray(np.tile(sinT, (2, 1))).astype(
        ml_dtypes.bfloat16
    )  # [128, CTX]
    cg = np.arange(P)[None, :]
    ii = np.arange(P)[:, None]
    tri = (cg >= ii).astype(np.float32).astype(ml_dtypes.bfloat16)  # keep q >= k
    ident = np.eye(P, dtype=np.float32).astype(ml_dtypes.bfloat16)
    return Rm, sin2, tri, ident


def _run(x, Wq, Wk, Wv, Wo):
    nc = _get_program()
    Rm, sin2, tri, ident = _host_tables()

    def fold(W):
        W2 = W.reshape(ED, NH, HD)
        return np.einsum("enh,gh->eng", W2, Rm).reshape(ED, NH * HD)

    bf = ml_dtypes.bfloat16
    Wq_f = fold(Wq).astype(bf)
    Wk_f = fold(Wk).astype(bf)
    Wv_b = Wv.astype(bf)
    Wo_b = Wo.astype(bf)
    xT_b = [np.ascontiguousarray(x[b].T).astype(bf) for b in range(BATCH)]

    in_maps = []
    for core in range(NCORES):
        b, g = core // 4, core % 4
        cs = slice(256 * g, 256 * (g + 1))
        in_maps.append(
            {
                "xT": xT_b[b],
                "wq": np.ascontiguousarray(Wq_f[:, cs]),
                "wk": np.ascontiguousarray(Wk_f[:, cs]),
                "wv": np.ascontiguousarray(Wv_b[:, cs]),
                "wo": np.ascontiguousarray(Wo_b[cs, :]),
                "sin2": sin2,
                "tri": tri,
                "ident": ident,
            }
        )
    return nc, in_maps


def kernel(x, Wq, Wk, Wv, Wo, bo):
    x = np.asarray(x, dtype=np.float32)
    nc, in_maps = _run(x, np.asarray(Wq, np.float32), np.asarray(Wk, np.float32),
                       np.asarray(Wv, np.float32), np.asarray(Wo, np.float32))
    res = run_bass_kernel_spmd(nc, in_maps, core_ids=list(range(NCORES)))
    out = np.zeros((BATCH, CTX, ED), np.float32)
    for core in range(NCORES):
        b = core // 4
        out[b] += res.results[core]["z"]
    out += np.asarray(bo, np.float32)[None, None, :]
    return out


# revision 7
# speedup vs baseline: 1.2636x; 1.0080x over previous
"""Causal multi-head attention with RoPE (faithful to reference's cos<-sin
overwrite bug) on 8 TRN2 NeuronCores.

Sharding: data parallel on batch (2) x tensor parallel on heads (4 groups of
4 heads) = 8 cores. Each core computes, for its (batch, 4 heads), flash-style
causal attention and its partial out-projection; the host sums the 4 partials
per batch and adds the bias.

Key structure (v2):
- RoPE's rotate-half is folded into Wq/Wk on the host; on-device rope is one
  elementwise multiply by a sin table.
- Scores are computed transposed (S^T[k, q]) per 128-row k-chunk over a
  1024-wide q supertile, exp'd without max subtraction (logits bounded), with
  a triangular mask multiply on the diagonal 128-block only.
- Optionally (USE_FP8) the rope multiply emits fp8e4 q/k, which are repacked
  by DMA into [32, 2, ctx] DoubleRow layout; score matmuls then run in
  MatmulPerfMode.DoubleRow at half cost.
- A/V runs in output-partition form: per 128-q tile, O[q, d] accumulates in
  PSUM over k-chunks with pt chunks as the stationary operand; an extra ones
  column in V yields softmax row-sums in the same pass. Normalization is a
  DVE reciprocal ([128,1]) + per-partition tensor_scalar multiply during the
  PSUM eviction.
- Normalized O tiles are pair-transposed on the PE (via identity) into
  [d, q] layout for the out-projection; z is staged in SBUF and written to
  DRAM two 128-row tiles per DMA.
"""

import contextlib

import numpy as np
import ml_dtypes

import concourse.bass as bass
import concourse.mybir as mybir
import concourse.tile as tile
from concourse.bass_utils import run_bass_kernel_spmd

BATCH, CTX, ED = 2, 2048, 1024
NH, HD = 16, 64
ROPE_BASE = 10000.0
P = 128
NCORES = 8
HPC = 4  # heads per core
QS = 1024  # q supertile width
NKT = CTX // P  # 16 k-chunks
NEC = ED // P  # 8 contraction chunks

F32 = mybir.dt.float32
BF16 = mybir.dt.bfloat16
FP8 = mybir.dt.float8e4

USE_FP8 = True  # fp8e4 DoubleRow score matmuls


def _split_multi_waits(nc, max_waits=1):
    """Walrus in this container rejects >1 sync wait per instruction; hoist
    extra waits onto preceding same-engine NoOps (semantically identical:
    engines execute their stream in order)."""
    n = 0
    for func in nc.m.functions:
        for bb in func.blocks:
            insts = list(bb.instructions)
            out = []
            changed = False
            for inst in insts:
                si = inst.sync_info
                if si and si.on_wait and len(si.on_wait) > max_waits:
                    waits = list(si.on_wait)
                    for k, w in enumerate(waits[:-max_waits]):
                        nop = mybir.InstNoOp(
                            name=f"{inst.name}-ws{k}",
                            sync_info=mybir.SyncInfo(on_wait=[w], on_update=[]),
                        )
                        nop.engine = inst.engine
                        out.append(nop)
                        n += 1
                    inst.sync_info = mybir.SyncInfo(
                        on_wait=waits[-max_waits:], on_update=list(si.on_update or [])
                    )
                    changed = True
                out.append(inst)
            if changed:
                bb.instructions = out
    return n


def _emit(nc, xT, wq, wk, wv, wo, sin2, tri, ident, z, tc):
    Exp = mybir.ActivationFunctionType.Exp
    Copy = mybir.ActivationFunctionType.Copy
    MULT = mybir.AluOpType.mult
    DR = mybir.MatmulPerfMode.DoubleRow

    with contextlib.ExitStack() as ctx:
        pers = ctx.enter_context(tc.tile_pool(name="pers", bufs=1))
        ptp = ctx.enter_context(tc.tile_pool(name="ptp", bufs=20))
        work = ctx.enter_context(tc.tile_pool(name="work", bufs=2))
        psum = ctx.enter_context(tc.tile_pool(name="psum", bufs=1, space="PSUM"))

        xt_sb = pers.tile([P, NEC, CTX], BF16, tag="xt")
        wq_sb = pers.tile([P, NEC, 256], BF16, tag="wq")
        wk_sb = pers.tile([P, NEC, 256], BF16, tag="wk")
        wv_sb = pers.tile([P, NEC, 256], BF16, tag="wv")
        wo_sb = pers.tile([P, 2, ED], BF16, tag="wo")
        sin_sb = pers.tile([P, CTX], BF16, tag="sin")
        tri_sb = pers.tile([P, P], BF16, tag="tri")
        id_sb = pers.tile([P, P], BF16, tag="id")
        v_sb = pers.tile([P, NKT, HPC, 66], BF16, tag="v")
        ot_sb = pers.tile([P, 2, CTX], BF16, tag="ot")
        if USE_FP8:
            q8p = pers.tile([P, 2, CTX], FP8, tag="q8p")
            k8p = pers.tile([P, 2, CTX], FP8, tag="k8p")
            q8 = pers.tile([32, HPC, 2, CTX], FP8, tag="q8")
            k8 = pers.tile([32, HPC, 2, CTX], FP8, tag="k8")
        else:
            qt_sb = pers.tile([P, 2, CTX], BF16, tag="qt")
            kt_sb = pers.tile([P, 2, CTX], BF16, tag="kt")

        # ---- loads: fine-grained at the front so projections start ASAP ----
        def ld(dst, src):
            nc.sync.dma_start(dst, src)

        ld(wq_sb[:, 0:4, :], wq[0:512, :].rearrange("(c p) n -> p c n", p=P))
        ld(xt_sb[:, 0:4, 0:512], xT[0:512, 0:512].rearrange("(c p) n -> p c n", p=P))
        ld(sin_sb[:], sin2)
        ld(wq_sb[:, 4:8, :], wq[512:1024, :].rearrange("(c p) n -> p c n", p=P))
        ld(xt_sb[:, 4:8, 0:512], xT[512:1024, 0:512].rearrange("(c p) n -> p c n", p=P))
        ld(wk_sb[:, 0:4, :], wk[0:512, :].rearrange("(c p) n -> p c n", p=P))
        ld(wk_sb[:, 4:8, :], wk[512:1024, :].rearrange("(c p) n -> p c n", p=P))
        ld(
            xt_sb[:, 0:4, 512:1024],
            xT[0:512, 512:1024].rearrange("(c p) n -> p c n", p=P),
        )
        ld(
            xt_sb[:, 4:8, 512:1024],
            xT[512:1024, 512:1024].rearrange("(c p) n -> p c n", p=P),
        )
        ld(tri_sb[:], tri)
        for c2 in range(2):
            ld(
                wv_sb[:, 4 * c2 : 4 * c2 + 4, :],
                wv[512 * c2 : 512 * (c2 + 1), :].rearrange("(c p) n -> p c n", p=P),
            )
        ld(
            xt_sb[:, :, 1024:1536],
            xT[:, 1024:1536].rearrange("(c p) n -> p c n", p=P),
        )
        ld(id_sb[:], ident)
        ld(
            xt_sb[:, :, 1536:2048],
            xT[:, 1536:2048].rearrange("(c p) n -> p c n", p=P),
        )
        ld(wo_sb[:], wo.rearrange("(cc p) n -> p cc n", p=P))
        nc.gpsimd.memset(v_sb[:, :, :, 64:65], 1.0)

        # ---- Q/K projection + rope for one (which, pair, supertile, half) ----
        def qk_proj(which, p, j, half, ptag="aux"):
            w_sb = wq_sb if which == "q" else wk_sb
            if ptag == "st":
                ps = psum.tile([P, QS], F32, tag="st", bufs=2, name="psqk")
            else:
                ps = psum.tile([P, ED], F32, tag="aux", bufs=1, name="psqk")
            c0 = j * QS + half * 512
            for c in range(NEC):
                nc.tensor.matmul(
                    ps[:, 0:512],
                    lhsT=w_sb[:, c, p * P : (p + 1) * P],
                    rhs=xt_sb[:, c, c0 : c0 + 512],
                    start=(c == 0),
                    stop=(c == NEC - 1),
                )
            if USE_FP8:
                dst = q8p if which == "q" else k8p
            else:
                dst = qt_sb if which == "q" else kt_sb
            nc.vector.tensor_tensor(
                out=dst[:, p, c0 : c0 + 512],
                in0=ps[:, 0:512],
                in1=sin_sb[:, c0 : c0 + 512],
                op=MULT,
            )
            if USE_FP8 and half == 1:
                src, pk = (q8p, q8) if which == "q" else (k8p, k8)
                for s in range(2):
                    for jj in range(2):
                        ld(
                            pk[:, 2 * p + s, jj, j * QS : (j + 1) * QS],
                            src[
                                s * HD + 32 * jj : s * HD + 32 * jj + 32,
                                p,
                                j * QS : (j + 1) * QS,
                            ],
                        )

        # ---- V projection (natural layout, per ctx tile) ----
        def v_proj(t):
            ps = psum.tile([P, 256], F32, tag="small", bufs=2)
            for c in range(NEC):
                nc.tensor.matmul(
                    ps[:],
                    lhsT=xt_sb[:, c, t * P : (t + 1) * P],
                    rhs=wv_sb[:, c, :],
                    start=(c == 0),
                    stop=(c == NEC - 1),
                )
            nc.vector.tensor_copy(
                out=v_sb[:, t, :, 0:64],
                in_=ps[:].rearrange("p (h d) -> p h d", h=HPC),
            )

        # ---- scores+exp(+mask) for one (head, supertile, k-chunk) ----
        def scores(h, j, KT):
            p, s = h // 2, h % 2
            st = psum.tile([P, QS], F32, tag="st", bufs=2)
            lo = max(KT * P - j * QS, 0)
            for half in (0, 1):
                a = max(lo, half * 512)
                b = (half + 1) * 512
                if a >= b:
                    continue
                if USE_FP8:
                    nc.tensor.matmul(
                        st[:, a:b],
                        lhsT=k8[:, h, :, KT * P : (KT + 1) * P],
                        rhs=q8[:, h, :, j * QS + a : j * QS + b],
                        start=True,
                        stop=True,
                        perf_mode=DR,
                    )
                else:
                    nc.tensor.matmul(
                        st[:, a:b],
                        lhsT=kt_sb[s * HD : (s + 1) * HD, p, KT * P : (KT + 1) * P],
                        rhs=qt_sb[s * HD : (s + 1) * HD, p, j * QS + a : j * QS + b],
                        start=True,
                        stop=True,
                    )
            pt = ptp.tile([P, QS], BF16, tag="pt")
            nc.scalar.activation(pt[:, lo:QS], st[:, lo:QS], Exp, scale=0.125)
            if KT >= 8 * j:
                d = KT * P - j * QS
                nc.gpsimd.tensor_tensor(
                    out=pt[:, d : d + P], in0=pt[:, d : d + P], in1=tri_sb[:], op=MULT
                )
            return pt, lo

        # ---- A/V for one (head, supertile, local q tile) in O[q,d] form ----
        def av(h, j, t, pts, osb_t):
            T = 8 * j + t
            o = psum.tile([P, 66], F32, tag="small", bufs=2)
            for KT in range(T + 1):
                pt, lo = pts[KT]
                off = t * P  # pt cols are q-local; cols [0:lo) never read
                nc.tensor.matmul(
                    o[:, 0:65],
                    lhsT=pt[:, off : off + P],
                    rhs=v_sb[:, KT, h, 0:65],
                    start=(KT == 0),
                    stop=(KT == T),
                )
            rc = work.tile([P, 1], F32, tag="rc", bufs=6)
            nc.vector.reciprocal(rc[:], o[:, 64:65])
            nc.vector.tensor_scalar(
                out=osb_t[:, h, :],
                in0=o[:, 0:64],
                scalar1=rc[:],
                scalar2=None,
                op0=MULT,
            )

        # ---- pair transpose of normalized O into [d, q] for out_proj ----
        def transpose_pair(cc, T, osb_t):
            tr = psum.tile([P, P], BF16, tag="small", bufs=2)
            nc.tensor.transpose(tr[:], osb_t[:, 2 * cc : 2 * cc + 2, :], id_sb[:])
            nc.vector.tensor_copy(out=ot_sb[:, cc, T * P : (T + 1) * P], in_=tr[:])

        # ---- out projection for one ctx tile; z staged 2 tiles per DMA ----
        zstage = {}

        def out_proj(T):
            zp = psum.tile([P, ED], F32, tag="aux", bufs=1)
            for cc in (0, 1):
                for nh in (0, 1):
                    nc.tensor.matmul(
                        zp[:, nh * 512 : (nh + 1) * 512],
                        lhsT=ot_sb[:, cc, T * P : (T + 1) * P],
                        rhs=wo_sb[:, cc, nh * 512 : (nh + 1) * 512],
                        start=(cc == 0),
                        stop=(cc == 1),
                    )
            if T % 2 == 0:
                zstage[T // 2] = work.tile([P, 2, ED], F32, tag="zs", bufs=2, name=f"zs{T}")
            zs_t = zstage[T // 2]
            nc.vector.tensor_copy(out=zs_t[:, T % 2, :], in_=zp[:])
            if T % 2 == 1:
                ld(
                    z[(T - 1) * P : (T + 1) * P, :].rearrange(
                        "(a p) n -> p a n", p=P
                    ),
                    zs_t[:],
                )

        # ---- emission schedule ----
        # Fillers are PE work units pulled between attention chunks so the
        # exp-feeding scores stay first in priority with dense PE work behind.
        fillers = []

        def pull_filler():
            if fillers:
                fillers.pop(0)()

        for p, half in ((0, 0), (0, 1)):
            qk_proj("q", p, 0, half, ptag="st")
            qk_proj("k", p, 0, half, ptag="st")

        osb = {}

        def run_supertile(j):
            nkt = 8 * (j + 1)
            for h in range(HPC):
                pts = {}
                for KT in range(nkt):
                    pull_filler()
                    pts[KT] = scores(h, j, KT)
                    t = KT - 8 * j
                    if t >= 0:
                        T = KT
                        if h == 0:
                            osb[T] = work.tile([P, HPC, 64], BF16, tag="osb", bufs=16, name=f"osb{T}")
                        av(h, j, t, pts, osb[T])
                        if h == 2:
                            transpose_pair(0, T, osb[T])
                        if h == 3:
                            transpose_pair(1, T, osb[T])
                            out_proj(T)

        # fillers consumed during supertile 0 (32 chunks): remaining j0
        # projections, all 16 v tiles, then the j1 q/k projections
        fillers.extend([(lambda t=t: v_proj(t)) for t in range(4)])
        for p, half in ((1, 0), (1, 1)):
            fillers.append(lambda p=p, half=half: qk_proj("q", p, 0, half))
            fillers.append(lambda p=p, half=half: qk_proj("k", p, 0, half))
        fillers.extend([(lambda t=t: v_proj(t)) for t in range(4, 16)])
        for p in (0, 1):
            for half in (0, 1):
                fillers.append(lambda p=p, half=half: qk_proj("q", p, 1, half))
                fillers.append(lambda p=p, half=half: qk_proj("k", p, 1, half))

        run_supertile(0)
        run_supertile(1)
        while fillers:
            pull_filler()


def _build_program(split_waits=True):
    nc = bass.Bass("TRN2", target_bir_lowering=False, debug=False, num_devices=NCORES)
    xT = nc.dram_tensor("xT", [ED, CTX], BF16, kind="ExternalInput").ap()
    wq = nc.dram_tensor("wq", [ED, 256], BF16, kind="ExternalInput").ap()
    wk = nc.dram_tensor("wk", [ED, 256], BF16, kind="ExternalInput").ap()
    wv = nc.dram_tensor("wv", [ED, 256], BF16, kind="ExternalInput").ap()
    wo = nc.dram_tensor("wo", [256, ED], BF16, kind="ExternalInput").ap()
    sin2 = nc.dram_tensor("sin2", [P, CTX], BF16, kind="ExternalInput").ap()
    tri = nc.dram_tensor("tri", [P, P], BF16, kind="ExternalInput").ap()
    ident = nc.dram_tensor("ident", [P, P], BF16, kind="ExternalInput").ap()
    z = nc.dram_tensor("z", [CTX, ED], F32, kind="ExternalOutput").ap()
    with tile.TileContext(nc) as tc:
        _emit(nc, xT, wq, wk, wv, wo, sin2, tri, ident, z, tc)
    if split_waits:
        _split_multi_waits(nc)
    return nc


_PROGRAM = None


def _get_program():
    global _PROGRAM
    if _PROGRAM is None:
        _PROGRAM = _build_program()
    return _PROGRAM


def _host_tables():
    # rotate-half fold matrix: q_rot = R q
    Rm = np.zeros((HD, HD), np.float32)
    for i in range(HD // 2):
        Rm[i, i] = 1.0
        Rm[i, i + 32] = -1.0
        Rm[i + 32, i + 32] = 1.0
        Rm[i + 32, i] = 1.0
    j = np.arange(HD // 2, dtype=np.float32)
    thetas = 1.0 / ROPE_BASE ** (2.0 * j / (HD // 2))
    pos = np.arange(CTX, dtype=np.float32)
    ang = pos[:, None] * thetas[None, :]
    sinT = np.sin(np.concatenate([ang, ang], axis=-1)).T.astype(np.float32)  # [64,CTX]
    sin2 = np.ascontiguousarray(np.tile(sinT, (2, 1))).astype(
        ml_dtypes.bfloat16
    )  # [128, CTX]
    cg = np.arange(P)[None, :]
    ii = np.arange(P)[:, None]
    tri = (cg >= ii).astype(np.float32).astype(ml_dtypes.bfloat16)  # keep q >= k
    ident = np.eye(P, dtype=np.float32).astype(ml_dtypes.bfloat16)
    return Rm, sin2, tri, ident


def _run(x, Wq, Wk, Wv, Wo):
    nc = _get_program()
    Rm, sin2, tri, ident = _host_tables()

    def fold(W):
        W2 = W.reshape(ED, NH, HD)
        return np.einsum("enh,gh->eng", W2, Rm).reshape(ED, NH * HD)

    bf = ml_dtypes.bfloat16
    Wq_f = fold(Wq).astype(bf)
    Wk_f = fold(Wk).astype(bf)
    Wv_b = Wv.astype(bf)
    Wo_b = Wo.astype(bf)
    xT_b = [np.ascontiguousarray(x[b].T).astype(bf) for b in range(BATCH)]

    in_maps = []
    for core in range(NCORES):
        b, g = core // 4, core % 4
        cs = slice(256 * g, 256 * (g + 1))
        in_maps.append(
            {
                "xT": xT_b[b],
                "wq": np.ascontiguousarray(Wq_f[:, cs]),
                "wk": np.ascontiguousarray(Wk_f[:, cs]),
                "wv": np.ascontiguousarray(Wv_b[:, cs]),
                "wo": np.ascontiguousarray(Wo_b[cs, :]),
                "sin2": sin2,
                "tri": tri,
                "ident": ident,
            }
        )
    return nc, in_maps


def kernel(x, Wq, Wk, Wv, Wo, bo):
    x = np.asarray(x, dtype=np.float32)
    nc, in_maps = _run(x, np.asarray(Wq, np.float32), np.asarray(Wk, np.float32),
                       np.asarray(Wv, np.float32), np.asarray(Wo, np.float32))
    res = run_bass_kernel_spmd(nc, in_maps, core_ids=list(range(NCORES)))
    out = np.zeros((BATCH, CTX, ED), np.float32)
    for core in range(NCORES):
        b = core // 4
        out[b] += res.results[core]["z"]
    out += np.asarray(bo, np.float32)[None, None, :]
    return out
